# revision 23
# baseline (speedup 1.0000x reference)
"""Trainium2 Bass kernel for a single-head AttentionBlock with residual.

Reference computation (per batch b):
    q = x @ Wq^T ; k = x @ Wk^T ; v = x @ Wv^T      (bq/bk/bv zero per spec)
    s = (q @ k^T) / sqrt(D)         [S, S]
    s = where(mask[b] == 0 (keys), -1e10, s)
    a = softmax(s, axis=-1)
    out = x + (a @ v) @ Wo^T + bo

Sharding: 8 cores = 4 batches x 2 query-halves (SQ=1024 rows each), no
collectives (the collectives core takes a fixed ~50us to boot, which
puts any K/V exchange on the critical path; cheaper to duplicate).

Key optimizations over the fp16 dense baseline:
 1. Weight fusion (host-side, exact f32 algebra): with a single head and
    square projections, q@k^T == x_q @ (Wq^T Wk) @ x_k^T and
    (a@v)@Wo^T == a @ (x_k @ (Wo Wv)^T).  The host precomputes
    Wg = Wq^T@Wk and Wvo = Wv^T@Wo^T once; the kernel then runs only
    TWO dense projections (G = x_q@Wg, V' = x_k@Wvo) instead of four,
    and the A@V pass directly yields the output rows.
 2. fp8 (e4m3) matmuls in DoubleRow perf mode: each matmul consumes two
    128-row contraction subtiles at once (2x PE throughput vs fp16).
    Scale bookkeeping: Wg/Wvo are pre-scaled x32 on the host so their
    entries sit in fp8's normal range; G is kept raw (std ~32), V' is
    rescaled /32 at the psum->fp8 cast, scores get exp(2^-10 * ps + mb)
    where mb also carries -6*ln2 so expt = 2^-6 * exp(s) stays in fp8
    range through the A@V accumulation.  (Dual-fp8 Ldweights requires
    the pair-dim byte stride to be 16B-aligned -> V' is padded to D+16.)
 3. Masked-key compaction: mask[b] knocks out ~half the keys; the host
    gathers the batch's unmasked keys (<=538 of 1024 per half for the
    spec inputs) into a padded [D, 2*KH=1152] block, and scores/A@V run
    over 1152 key slots instead of 2048. Pad slots get bias -30000 ->
    exp == 0. A 10th all-zero key tile keeps the A@V loop in pure
    DoubleRow pairs (a lone odd tile would run at half throughput).
 4. Chunk-outer compute (scores -> per-query-tile rowsum/A@V/store) so
    the normalize+stores drain while the other chunk is still on the
    tensor engine.

Row sums ride along in the A@V pass via a ones column appended to V'
(5 tiny DoubleRow matmuls into a [q, 1] psum column), which lands
per-query scalars directly in the output-tile partition layout — no
transposes.  The reciprocal+normalize+residual run on the vector
engine.

Softmax max-subtraction is skipped: scores are ~N(0,1) here, exp < ~200,
and the 2^-6 rescale keeps everything comfortably inside fp8/fp32.

bq/bk are assumed zero (spec fill=zeros); nonzero or a mask half-count
above KH triggers an exact numpy fallback (never hit for the spec
inputs). bv/bo are folded into the residual on the host (exact).
"""

import functools
from contextlib import ExitStack

import numpy as np
import ml_dtypes

import concourse.bass as bass
import concourse.tile as tile
from concourse import bacc, mybir
from concourse.bass_utils import run_bass_kernel_spmd

P = 128
NEG_BIAS = -30000.0
N_CORES = 8
KH = 576                 # per-half compacted key capacity (4.5 tiles)
WSCALE = 32.0            # host pre-scale on the fused weight matrices
EXP_OFF = -6.0 * float(np.log(2.0))   # expt = 2^-6 * exp(s)
FP8 = ml_dtypes.float8_e4m3fn


def _chunks(total, size):
    return [(o, min(size, total - o)) for o in range(0, total, size)]


def build_program(D=1024, SQ=1024, kh=KH, n_cores=8):
    """Build + compile the single-core Bass program (same program on all cores)."""
    f32 = mybir.dt.float32
    f16 = mybir.dt.float16
    fp8 = mybir.dt.float8e4
    DT = D // P    # d contraction tiles
    SK = 2 * kh            # compacted key slots (1152)
    KTc = SK // P          # real key tiles (9)
    KTp = KTc + (KTc % 2)  # padded to even (10) for pure DoubleRow A@V
    QT = SQ // P   # query row tiles
    DR = mybir.MatmulPerfMode.DoubleRow

    nc = bacc.Bacc("TRN2", target_bir_lowering=False, debug=False,
                   num_devices=n_cores)

    xqt_d = nc.dram_tensor("xqt", [D, SQ], fp8, kind="ExternalInput")
    xkt_d = nc.dram_tensor("xkt", [D, SK], fp8, kind="ExternalInput")
    hs_d = nc.dram_tensor("hs", [SQ, D], f16, kind="ExternalInput")
    wg_d = nc.dram_tensor("wg", [D, D], fp8, kind="ExternalInput")
    wvo_d = nc.dram_tensor("wvo", [D, D], fp8, kind="ExternalInput")
    mb_d = nc.dram_tensor("mb", [P, KTc], f32, kind="ExternalInput")
    out_d = nc.dram_tensor("out", [SQ, D], f16, kind="ExternalOutput")

    Exp = mybir.ActivationFunctionType.Exp
    Copy = mybir.ActivationFunctionType.Copy
    mult = mybir.AluOpType.mult
    add = mybir.AluOpType.add

    with tile.TileContext(nc) as tc, ExitStack() as ctx:
        bigA = ctx.enter_context(tc.tile_pool(name="bigA", bufs=1))
        qk_pool = ctx.enter_context(tc.tile_pool(name="qk", bufs=1))
        v_pool = ctx.enter_context(tc.tile_pool(name="vp", bufs=1))
        wpool = ctx.enter_context(tc.tile_pool(name="w", bufs=2))
        con = ctx.enter_context(tc.tile_pool(name="const", bufs=1))
        outp = ctx.enter_context(tc.tile_pool(name="outs", bufs=2))

        pp = ctx.enter_context(tc.tile_pool(name="pp", bufs=6, space="PSUM"))
        rsp = ctx.enter_context(tc.tile_pool(name="rsp", bufs=2, space="PSUM"))

        # ---- PE warmup during the initial DMA wait (HAM ramp) ----
        ones1h = con.tile([1, 1], fp8)
        nc.vector.memset(ones1h[:], 1.0)
        warm_in = con.tile([1, 256], fp8)
        nc.vector.memset(warm_in[:], 0.0)
        warm_ps = pp.tile([P, 512], f32, tag="pp")
        N_WARM = 16
        for i in range(N_WARM):
            nc.tensor.matmul(warm_ps[:1, :256], ones1h[:], warm_in[:],
                             start=(i == 0), stop=(i == N_WARM - 1))
        warm_out = con.tile([1, 256], f32)
        nc.vector.tensor_copy(warm_out[:], warm_ps[:1, :256])

        gt = qk_pool.tile([P, DT, SQ], fp8, tag="gt")
        # free width D+16 keeps the DoubleRow pair-dim stride 16B-aligned
        # (dual-fp8 Ldweights ISA restriction); col D is the ones column.
        vp = v_pool.tile([P, KTp, D + 16], fp8, tag="v")
        expt = bigA.tile([P, KTp, SQ], fp8, tag="expt")

        _engs = [nc.gpsimd, nc.sync, nc.scalar]

        def load_w(dram_t, eng=None, split=1):
            w = wpool.tile([P, DT, D], fp8, tag="w")
            wv_ = dram_t.ap().rearrange("(t p) e -> p t e", p=P)
            split = min(split, DT)
            assert DT % split == 0, (DT, split)
            step = DT // split
            for i in range(split):
                e = _engs[i % 3] if eng is None else eng
                sl = slice(i * step, (i + 1) * step)
                e.dma_start(w[:, sl, :], wv_[:, sl, :])
            return w

        # first-needed loads first: wg + xqt gate the G projection.
        # dt-pair i of the projection needs only slice i of each tensor, and
        # tile-granular dependency tracking would stall the first matmul on
        # the whole load — so each pair slice gets its OWN tile, interleaved
        # across the three DMA-capable queues so pair 0 lands first.
        xqt_v = xqt_d.ap().rearrange("(t p) q -> p t q", p=P)
        wg_v = wg_d.ap().rearrange("(t p) e -> p t e", p=P)
        xkt_v = xkt_d.ap().rearrange("(t p) k -> p t k", p=P)
        wgp = [wpool.tile([P, 2, D], fp8, tag=f"wg{i}", name=f"wgp{i}")
               for i in range(4)]
        xqp = [bigA.tile([P, 2, SQ], fp8, tag=f"xq{i}", name=f"xqp{i}")
               for i in range(4)]
        xkp = [con.tile([P, 2, SK], fp8, name=f"xkp{i}")
               for i in range(4)]
        qs = [nc.gpsimd, nc.sync, nc.scalar]
        qi_ = 0
        for i in range(4):
            sl = slice(2 * i, 2 * i + 2)
            qs[qi_ % 3].dma_start(wgp[i][:], wg_v[:, sl, :]); qi_ += 1
            qs[qi_ % 3].dma_start(xqp[i][:], xqt_v[:, sl, :]); qi_ += 1
        for i in range(4):
            sl = slice(2 * i, 2 * i + 2)
            qs[qi_ % 3].dma_start(xkp[i][:], xkt_v[:, sl, :]); qi_ += 1
        wvo = load_w(wvo_d, nc.gpsimd)
        # residual rows, preloaded once (read late by the normalize)
        hst_all = con.tile([P, QT, D], f16)
        hs_v = hs_d.ap().rearrange("(t p) f -> p t f", p=P)
        nc.sync.dma_start(hst_all[:, 0:QT // 2, :], hs_v[:, 0:QT // 2, :])
        nc.sync.dma_start(hst_all[:, QT // 2:QT, :], hs_v[:, QT // 2:QT, :])
        # constants + zero-fills, behind the critical loads on their queues
        mb = con.tile([P, KTc], f32)
        nc.gpsimd.dma_start(mb[:], mb_d.ap())
        nc.gpsimd.memset(vp[:, :, D:D + 1], 1.0)  # ones col -> row sums
        if KTp != KTc:  # zero the padding key tile (never written otherwise)
            nc.gpsimd.memset(vp[:, KTc, 0:D], 0.0)
            nc.vector.memset(expt[:, KTc, :], 0.0)

        # ---- G[q-dim e', q] = wg.T-rows contracted with xqt, DoubleRow ----
        ci = 0
        for et in range(DT):
            for qo, qn in _chunks(SQ, 512):
                ps = pp.tile([P, 512], f32, tag="pp")
                for t in range(DT // 2):
                    nc.tensor.matmul(
                        ps[:, :qn],
                        wgp[t][:, :, et * P:(et + 1) * P],
                        xqp[t][:, :, qo:qo + qn],
                        start=(t == 0), stop=(t == DT // 2 - 1),
                        perf_mode=DR,
                    )
                if ci % 2:
                    nc.vector.tensor_copy(gt[:, et, qo:qo + qn], ps[:, :qn])
                else:
                    nc.scalar.activation(gt[:, et, qo:qo + qn], ps[:, :qn],
                                         Copy)
                ci += 1

        # ---- V'[k, f] = (xkt.T @ wvo)/32 ----
        for vt in range(KTc):
            for eo, en in _chunks(D, 512):
                ps = pp.tile([P, 512], f32, tag="pp")
                for t in range(DT // 2):
                    nc.tensor.matmul(
                        ps[:, :en], xkp[t][:, :, vt * P:(vt + 1) * P],
                        wvo[:, 2 * t:2 * t + 2, eo:eo + en],
                        start=(t == 0), stop=(t == DT // 2 - 1),
                        perf_mode=DR,
                    )
                if ci % 2:
                    nc.vector.tensor_scalar_mul(vp[:, vt, eo:eo + en],
                                                ps[:, :en], 1.0 / WSCALE)
                else:
                    nc.scalar.activation(vp[:, vt, eo:eo + en], ps[:, :en],
                                         Copy, bias=0.0, scale=1.0 / WSCALE)
                ci += 1

        # ---- main compute, chunk-outer so stores drain early ----
        rinv = con.tile([P, QT], f32)
        out_v = out_d.ap().rearrange("(t p) f -> t p f", p=P)
        out_engs = [nc.sync, nc.gpsimd]
        KP = KTp // 2  # DoubleRow pairs over the padded key tiles

        for qi, (qo, qn) in enumerate(_chunks(SQ, 512)):
            # scores^T + exp: expT[k, q] = 2^-6 exp(xkt.T@G^T * 2^-10 + mask)
            for kt_ in range(KTc):
                ps = pp.tile([P, 512], f32, tag="pp")
                for t in range(DT // 2):
                    nc.tensor.matmul(
                        ps[:, :qn],
                        xkp[t][:, :, kt_ * P:(kt_ + 1) * P],
                        gt[:, 2 * t:2 * t + 2, qo:qo + qn],
                        start=(t == 0), stop=(t == DT // 2 - 1),
                        perf_mode=DR,
                    )
                nc.scalar.activation(
                    expt[:, kt_, qo:qo + qn], ps[:, :qn], Exp,
                    bias=mb[:, kt_:kt_ + 1], scale=float(2.0 ** -10),
                )
            # per query-row-tile: rowsum column, then A@V -> normalize+store
            TPC = QT * qn // SQ
            for ti in range(TPC):
                qt_ = qi * TPC + ti
                rs = rsp.tile([P, 1], f32, tag="rs")
                for t in range(KP):
                    nc.tensor.matmul(
                        rs[:, :], expt[:, 2 * t:2 * t + 2,
                                       qt_ * P:(qt_ + 1) * P],
                        vp[:, 2 * t:2 * t + 2, D:D + 1],
                        start=(t == 0), stop=(t == KP - 1), perf_mode=DR,
                    )
                nc.vector.reciprocal(rinv[:, qt_:qt_ + 1], rs[:, :])
                outt = outp.tile([P, D], f16, tag="outt")
                for fo, fn in _chunks(D, 512):
                    ps = pp.tile([P, 512], f32, tag="pp")
                    for t in range(KP):
                        nc.tensor.matmul(
                            ps[:, :fn],
                            expt[:, 2 * t:2 * t + 2, qt_ * P:(qt_ + 1) * P],
                            vp[:, 2 * t:2 * t + 2, fo:fo + fn],
                            start=(t == 0), stop=(t == KP - 1),
                            perf_mode=DR,
                        )
                    nc.vector.scalar_tensor_tensor(
                        outt[:, fo:fo + fn], ps[:, :fn],
                        rinv[:, qt_:qt_ + 1],
                        hst_all[:, qt_, fo:fo + fn], op0=mult, op1=add,
                    )
                out_engs[qt_ % 2].dma_start(out_v[qt_], outt[:])

    nc.compile()
    return nc


@functools.lru_cache(maxsize=2)
def _get_program(D, SQ):
    return build_program(D, SQ)


def _numpy_reference(hidden_states, mask, Wq, bq, Wk, bk, Wv, bv, Wo, bo):
    """Exact fallback (used only if bq/bk nonzero or mask counts exceed KH)."""
    x = hidden_states.astype(np.float64)
    q = x @ Wq.T.astype(np.float64) + bq
    k = x @ Wk.T.astype(np.float64) + bk
    v = x @ Wv.T.astype(np.float64) + bv
    s = np.einsum("bqd,bkd->bqk", q, k) / np.sqrt(x.shape[-1])
    s = np.where(mask[:, None, :] == 0, -1e10, s)
    s -= s.max(axis=-1, keepdims=True)
    e = np.exp(s)
    a = e / e.sum(axis=-1, keepdims=True)
    hid = np.einsum("bqk,bkd->bqd", a, v)
    out = x + hid @ Wo.T.astype(np.float64) + bo
    return out.astype(np.float32)


def make_in_maps(hidden_states, mask, Wq, bq, Wk, bk, Wv, bv, Wo, bo):
    hs = np.asarray(hidden_states, dtype=np.float32)
    mask = np.asarray(mask)
    B, S, D = hs.shape
    SQ = S // 2
    KTc = (2 * KH) // P

    # fused weights (exact f32 algebra, done once on the host):
    #   scores = x_q @ (Wq^T Wk) @ x_k^T ;  (a@v)@Wo^T = a @ (x_k @ Wv^T Wo^T)
    Wg = np.asarray(Wq, np.float32).T @ np.asarray(Wk, np.float32)
    Wvo = np.asarray(Wv, np.float32).T @ np.asarray(Wo, np.float32).T
    wg8 = np.ascontiguousarray(Wg * WSCALE).astype(FP8)
    wvo8 = np.ascontiguousarray(Wvo * WSCALE).astype(FP8)
    # v-bias and o-bias act as a constant shift after the output projection:
    # fold them into the residual input (exact).
    extra = (np.asarray(Wo, np.float32) @ np.asarray(bv, np.float32)
             + np.asarray(bo, np.float32))

    # per-(batch,half) compacted key indices
    idxs = {}
    for b in range(B):
        for h in range(2):
            idx = np.nonzero(mask[b, h * SQ:(h + 1) * SQ])[0]
            if len(idx) > KH:
                return None  # caller falls back to numpy
            idxs[(b, h)] = idx

    # per-batch compacted key block + bias (shared by the two pair cores)
    xkts, mbs = {}, {}
    for b in range(B):
        x8 = hs[b].astype(FP8)
        xkT = np.zeros((D, 2 * KH), FP8)
        bias = np.full(2 * KH, np.float32(NEG_BIAS))
        for h in range(2):
            idx = idxs[(b, h)]
            xkT[:, h * KH:h * KH + len(idx)] = x8[h * SQ + idx].T
            bias[h * KH:h * KH + len(idx)] = 0.0
        bias += np.float32(EXP_OFF)
        xkts[b] = xkT
        mbs[b] = np.ascontiguousarray(bias.reshape(KTc, P).T.astype(np.float32))

    in_maps = []
    for c in range(N_CORES):
        b, h = divmod(c, 2)
        xb = hs[b]
        x8 = xb.astype(FP8)
        xqT = np.ascontiguousarray(x8[h * SQ:(h + 1) * SQ].T)
        hsc = np.ascontiguousarray(
            (xb[h * SQ:(h + 1) * SQ] + extra[None, :]).astype(np.float16))
        in_maps.append(dict(xqt=xqT, xkt=xkts[b], hs=hsc, wg=wg8,
                            wvo=wvo8, mb=mbs[b]))
    return in_maps


def assemble_output(results, B, S, D):
    SQ = S // 2
    out = np.empty((B, S, D), np.float32)
    for c in range(N_CORES):
        b, h = divmod(c, 2)
        out[b, h * SQ:(h + 1) * SQ, :] = results[c]["out"].astype(np.float32)
    return out


def kernel(hidden_states, mask, Wq, bq, Wk, bk, Wv, bv, Wo, bo):
    hs = np.asarray(hidden_states, dtype=np.float32)
    B, S, D = hs.shape
    args = dict(hidden_states=hs, mask=np.asarray(mask),
                Wq=np.asarray(Wq, np.float32), bq=np.asarray(bq, np.float32),
                Wk=np.asarray(Wk, np.float32), bk=np.asarray(bk, np.float32),
                Wv=np.asarray(Wv, np.float32), bv=np.asarray(bv, np.float32),
                Wo=np.asarray(Wo, np.float32), bo=np.asarray(bo, np.float32))
    if np.any(args["bq"]) or np.any(args["bk"]) or (S, D) != (2048, 1024):
        return _numpy_reference(**args)

    in_maps = make_in_maps(**args)
    if in_maps is None:
        return _numpy_reference(**args)
    nc = _get_program(D, S // 2)
    res = run_bass_kernel_spmd(nc, in_maps, core_ids=list(range(N_CORES)))
    return assemble_output(res.results, B, S, D)


if __name__ == "__main__":
    rng = np.random.default_rng(0)
    B, S, D = 4, 2048, 1024
    ins = dict(
        hidden_states=rng.standard_normal((B, S, D)).astype(np.float32),
        mask=rng.integers(0, 2, (B, S)).astype(np.int32),
        Wq=(rng.standard_normal((D, D)) / np.sqrt(D)).astype(np.float32),
        bq=np.zeros(D, np.float32),
        Wk=(rng.standard_normal((D, D)) / np.sqrt(D)).astype(np.float32),
        bk=np.zeros(D, np.float32),
        Wv=(rng.standard_normal((D, D)) / np.sqrt(D)).astype(np.float32),
        bv=np.zeros(D, np.float32),
        Wo=(rng.standard_normal((D, D)) / np.sqrt(D)).astype(np.float32),
        bo=np.zeros(D, np.float32),
    )
    out = kernel(**ins)
    ref = _numpy_reference(**ins)
    err = np.max(np.abs(out - ref)) / np.max(np.abs(ref))
    print("rel err vs numpy:", err)


# revision 25
# speedup vs baseline: 1.1625x; 1.1625x over previous
"""Trainium2 Bass kernel for a single-head AttentionBlock with residual.

Reference computation (per batch b):
    q = x @ Wq^T ; k = x @ Wk^T ; v = x @ Wv^T      (bq/bk/bv zero per spec)
    s = (q @ k^T) / sqrt(D)         [S, S]
    s = where(mask[b] == 0 (keys), -1e10, s)
    a = softmax(s, axis=-1)
    out = x + (a @ v) @ Wo^T + bo

Sharding: 8 cores = 4 batches x 2 query-halves (SQ=1024 rows each), no
collectives (the collectives core takes a fixed ~50us to boot, which
puts any K/V exchange on the critical path; cheaper to duplicate).

Key optimizations over the fp16 dense baseline:
 1. Weight fusion (host-side, exact f32 algebra): with a single head and
    square projections, q@k^T == x_q @ (Wq^T Wk) @ x_k^T and
    (a@v)@Wo^T == a @ (x_k @ (Wo Wv)^T).  The host precomputes
    Wg = Wq^T@Wk and Wvo = Wv^T@Wo^T once; the kernel then runs only
    TWO dense projections (G = x_q@Wg, V' = x_k@Wvo) instead of four,
    and the A@V pass directly yields the output rows.
 2. fp8 (e4m3) matmuls in DoubleRow perf mode: each matmul consumes two
    128-row contraction subtiles at once (2x PE throughput vs fp16).
    Scale bookkeeping: Wg/Wvo are pre-scaled x32 on the host so their
    entries sit in fp8's normal range; G is kept raw (std ~32), V' is
    rescaled /32 at the psum->fp8 cast, scores get exp(2^-10 * ps + mb)
    where mb also carries -6*ln2 so expt = 2^-6 * exp(s) stays in fp8
    range through the A@V accumulation.  (Dual-fp8 Ldweights requires
    the pair-dim byte stride to be 16B-aligned -> V' is padded to D+16.)
 3. Masked-key compaction: mask[b] knocks out ~half the keys; the host
    gathers the batch's unmasked keys (<=538 of 1024 per half for the
    spec inputs) into a padded [D, 2*KH=1152] block, and scores/A@V run
    over 1152 key slots instead of 2048. Pad slots get bias -30000 ->
    exp == 0. A 10th all-zero key tile keeps the A@V loop in pure
    DoubleRow pairs (a lone odd tile would run at half throughput).
 4. Chunk-outer compute (scores -> per-query-tile rowsum/A@V/store) so
    the normalize+stores drain while the other chunk is still on the
    tensor engine.

Row sums ride along in the A@V pass via a ones column appended to V'
(5 tiny DoubleRow matmuls into a [q, 1] psum column), which lands
per-query scalars directly in the output-tile partition layout — no
transposes.  The reciprocal+normalize+residual run on the vector
engine.

Softmax max-subtraction is skipped: scores are ~N(0,1) here, exp < ~200,
and the 2^-6 rescale keeps everything comfortably inside fp8/fp32.

bq/bk are assumed zero (spec fill=zeros); nonzero or a mask half-count
above KH triggers an exact numpy fallback (never hit for the spec
inputs). bv/bo are folded into the residual on the host (exact).
"""

import functools
from contextlib import ExitStack

import numpy as np
import ml_dtypes

import concourse.bass as bass
import concourse.tile as tile
from concourse import bacc, mybir
from concourse.bass_utils import run_bass_kernel_spmd

P = 128
NEG_BIAS = -30000.0
N_CORES = 8
KH = 576                 # per-half compacted key capacity (4.5 tiles)
WSCALE = 32.0            # host pre-scale on the fused weight matrices
EXP_OFF = -6.0 * float(np.log(2.0))   # expt = 2^-6 * exp(s)
FP8 = ml_dtypes.float8_e4m3fn


def _chunks(total, size):
    return [(o, min(size, total - o)) for o in range(0, total, size)]


def build_program(D=1024, SQ=1024, kh=KH, n_cores=8):
    """Build + compile the single-core Bass program (same program on all cores)."""
    f32 = mybir.dt.float32
    f16 = mybir.dt.float16
    fp8 = mybir.dt.float8e4
    DT = D // P    # d contraction tiles
    SK = 2 * kh            # compacted key slots (1152)
    KTc = SK // P          # real key tiles (9)
    KTp = KTc + (KTc % 2)  # padded to even (10) for pure DoubleRow A@V
    QT = SQ // P   # query row tiles
    DR = mybir.MatmulPerfMode.DoubleRow

    nc = bacc.Bacc("TRN2", target_bir_lowering=False, debug=False,
                   num_devices=n_cores)

    xqt_d = nc.dram_tensor("xqt", [D, SQ], fp8, kind="ExternalInput")
    xkt_d = nc.dram_tensor("xkt", [D, SK], fp8, kind="ExternalInput")
    hs_d = nc.dram_tensor("hs", [SQ, D], f16, kind="ExternalInput")
    wg_d = nc.dram_tensor("wg", [D, D], fp8, kind="ExternalInput")
    wvo_d = nc.dram_tensor("wvo", [D, D], fp8, kind="ExternalInput")
    mb_d = nc.dram_tensor("mb", [P, KTc], f32, kind="ExternalInput")
    out_d = nc.dram_tensor("out", [SQ, D], f16, kind="ExternalOutput")

    Exp = mybir.ActivationFunctionType.Exp
    Copy = mybir.ActivationFunctionType.Copy
    mult = mybir.AluOpType.mult
    add = mybir.AluOpType.add

    with tile.TileContext(nc) as tc, ExitStack() as ctx:
        bigA = ctx.enter_context(tc.tile_pool(name="bigA", bufs=1))
        qk_pool = ctx.enter_context(tc.tile_pool(name="qk", bufs=1))
        v_pool = ctx.enter_context(tc.tile_pool(name="vp", bufs=1))
        wpool = ctx.enter_context(tc.tile_pool(name="w", bufs=2))
        con = ctx.enter_context(tc.tile_pool(name="const", bufs=1))
        outp = ctx.enter_context(tc.tile_pool(name="outs", bufs=2))

        pp = ctx.enter_context(tc.tile_pool(name="pp", bufs=6, space="PSUM"))
        rsp = ctx.enter_context(tc.tile_pool(name="rsp", bufs=2, space="PSUM"))

        # ---- PE warmup during the initial DMA wait (HAM ramp) ----
        ones1h = con.tile([1, 1], fp8)
        nc.vector.memset(ones1h[:], 1.0)
        warm_in = con.tile([1, 256], fp8)
        nc.vector.memset(warm_in[:], 0.0)
        warm_ps = pp.tile([P, 512], f32, tag="pp")
        N_WARM = 16
        for i in range(N_WARM):
            nc.tensor.matmul(warm_ps[:1, :256], ones1h[:], warm_in[:],
                             start=(i == 0), stop=(i == N_WARM - 1))
        warm_out = con.tile([1, 256], f32)
        nc.vector.tensor_copy(warm_out[:], warm_ps[:1, :256])

        gt = qk_pool.tile([P, DT, SQ], fp8, tag="gt")
        # free width D+16 keeps the DoubleRow pair-dim stride 16B-aligned
        # (dual-fp8 Ldweights ISA restriction); col D is the ones column.
        vp = v_pool.tile([P, KTp, D + 16], fp8, tag="v")
        expt = bigA.tile([P, KTp, SQ], fp8, tag="expt")

        _engs = [nc.gpsimd, nc.sync, nc.scalar]

        def load_w(dram_t, eng=None, split=1):
            w = wpool.tile([P, DT, D], fp8, tag="w")
            wv_ = dram_t.ap().rearrange("(t p) e -> p t e", p=P)
            split = min(split, DT)
            assert DT % split == 0, (DT, split)
            step = DT // split
            for i in range(split):
                e = _engs[i % 3] if eng is None else eng
                sl = slice(i * step, (i + 1) * step)
                e.dma_start(w[:, sl, :], wv_[:, sl, :])
            return w

        # first-needed loads first: wg + xqt gate the G projection
        xqt = bigA.tile([P, DT, SQ], fp8, tag="xqt")
        xqt_v = xqt_d.ap().rearrange("(t p) q -> p t q", p=P)
        wg = wpool.tile([P, DT, D], fp8, tag="w")
        wg_v = wg_d.ap().rearrange("(t p) e -> p t e", p=P)
        xkt = con.tile([P, DT, SK], fp8)
        xkt_v = xkt_d.ap().rearrange("(t p) k -> p t k", p=P)
        qs = [nc.gpsimd, nc.sync, nc.scalar]
        qi_ = 0
        for i in range(4):
            sl = slice(2 * i, 2 * i + 2)
            qs[qi_ % 3].dma_start(wg[:, sl, :], wg_v[:, sl, :]); qi_ += 1
            qs[qi_ % 3].dma_start(xqt[:, sl, :], xqt_v[:, sl, :]); qi_ += 1
        for i in range(4):
            sl = slice(2 * i, 2 * i + 2)
            qs[qi_ % 3].dma_start(xkt[:, sl, :], xkt_v[:, sl, :]); qi_ += 1
        wvo = load_w(wvo_d, nc.gpsimd)
        # residual rows, preloaded once (read late by the normalize)
        hst_all = con.tile([P, QT, D], f16)
        hs_v = hs_d.ap().rearrange("(t p) f -> p t f", p=P)
        nc.sync.dma_start(hst_all[:, 0:QT // 2, :], hs_v[:, 0:QT // 2, :])
        nc.sync.dma_start(hst_all[:, QT // 2:QT, :], hs_v[:, QT // 2:QT, :])
        # constants + zero-fills, behind the critical loads on their queues
        mb = con.tile([P, KTc], f32)
        nc.gpsimd.dma_start(mb[:], mb_d.ap())
        nc.gpsimd.memset(vp[:, :, D:D + 1], 1.0)  # ones col -> row sums
        if KTp != KTc:  # zero the padding key tile (never written otherwise)
            nc.gpsimd.memset(vp[:, KTc, 0:D], 0.0)
            nc.vector.memset(expt[:, KTc, :], 0.0)

        # ---- G[q-dim e', q] = wg.T-rows contracted with xqt, DoubleRow ----
        ci = 0
        for et in range(DT):
            for qo, qn in _chunks(SQ, 512):
                ps = pp.tile([P, 512], f32, tag="pp")
                for t in range(DT // 2):
                    nc.tensor.matmul(
                        ps[:, :qn],
                        wg[:, 2 * t:2 * t + 2, et * P:(et + 1) * P],
                        xqt[:, 2 * t:2 * t + 2, qo:qo + qn],
                        start=(t == 0), stop=(t == DT // 2 - 1),
                        perf_mode=DR,
                    )
                if ci % 2:
                    nc.vector.tensor_copy(gt[:, et, qo:qo + qn], ps[:, :qn])
                else:
                    nc.scalar.activation(gt[:, et, qo:qo + qn], ps[:, :qn],
                                         Copy)
                ci += 1

        # ---- V'[k, f] = (xkt.T @ wvo)/32 ----
        for vt in range(KTc):
            for eo, en in _chunks(D, 512):
                ps = pp.tile([P, 512], f32, tag="pp")
                for t in range(DT // 2):
                    nc.tensor.matmul(
                        ps[:, :en],
                        xkt[:, 2 * t:2 * t + 2, vt * P:(vt + 1) * P],
                        wvo[:, 2 * t:2 * t + 2, eo:eo + en],
                        start=(t == 0), stop=(t == DT // 2 - 1),
                        perf_mode=DR,
                    )
                if ci % 2:
                    nc.vector.tensor_scalar_mul(vp[:, vt, eo:eo + en],
                                                ps[:, :en], 1.0 / WSCALE)
                else:
                    nc.scalar.activation(vp[:, vt, eo:eo + en], ps[:, :en],
                                         Copy, bias=0.0, scale=1.0 / WSCALE)
                ci += 1

        # ---- main compute, chunk-outer so stores drain early ----
        rinv = con.tile([P, QT], f32)
        out_v = out_d.ap().rearrange("(t p) f -> t p f", p=P)
        out_engs = [nc.sync, nc.gpsimd]
        KP = KTp // 2  # DoubleRow pairs over the padded key tiles

        for qi, (qo, qn) in enumerate(_chunks(SQ, 512)):
            # scores^T + exp: expT[k, q] = 2^-6 exp(xkt.T@G^T * 2^-10 + mask)
            for kt_ in range(KTc):
                ps = pp.tile([P, 512], f32, tag="pp")
                for t in range(DT // 2):
                    nc.tensor.matmul(
                        ps[:, :qn],
                        xkt[:, 2 * t:2 * t + 2, kt_ * P:(kt_ + 1) * P],
                        gt[:, 2 * t:2 * t + 2, qo:qo + qn],
                        start=(t == 0), stop=(t == DT // 2 - 1),
                        perf_mode=DR,
                    )
                nc.scalar.activation(
                    expt[:, kt_, qo:qo + qn], ps[:, :qn], Exp,
                    bias=mb[:, kt_:kt_ + 1], scale=float(2.0 ** -10),
                )
            # per query-row-tile: rowsum column, then A@V -> normalize+store
            TPC = QT * qn // SQ
            for ti in range(TPC):
                qt_ = qi * TPC + ti
                rs = rsp.tile([P, 1], f32, tag="rs")
                for t in range(KP):
                    nc.tensor.matmul(
                        rs[:, :], expt[:, 2 * t:2 * t + 2,
                                       qt_ * P:(qt_ + 1) * P],
                        vp[:, 2 * t:2 * t + 2, D:D + 1],
                        start=(t == 0), stop=(t == KP - 1), perf_mode=DR,
                    )
                nc.vector.reciprocal(rinv[:, qt_:qt_ + 1], rs[:, :])
                outt = outp.tile([P, D], f16, tag="outt")
                for fo, fn in _chunks(D, 512):
                    ps = pp.tile([P, 512], f32, tag="pp")
                    for t in range(KP):
                        nc.tensor.matmul(
                            ps[:, :fn],
                            expt[:, 2 * t:2 * t + 2, qt_ * P:(qt_ + 1) * P],
                            vp[:, 2 * t:2 * t + 2, fo:fo + fn],
                            start=(t == 0), stop=(t == KP - 1),
                            perf_mode=DR,
                        )
                    nc.vector.scalar_tensor_tensor(
                        outt[:, fo:fo + fn], ps[:, :fn],
                        rinv[:, qt_:qt_ + 1],
                        hst_all[:, qt_, fo:fo + fn], op0=mult, op1=add,
                    )
                out_engs[qt_ % 2].dma_start(out_v[qt_], outt[:])

    nc.compile()
    return nc


@functools.lru_cache(maxsize=2)
def _get_program(D, SQ):
    return build_program(D, SQ)


def _numpy_reference(hidden_states, mask, Wq, bq, Wk, bk, Wv, bv, Wo, bo):
    """Exact fallback (used only if bq/bk nonzero or mask counts exceed KH)."""
    x = hidden_states.astype(np.float64)
    q = x @ Wq.T.astype(np.float64) + bq
    k = x @ Wk.T.astype(np.float64) + bk
    v = x @ Wv.T.astype(np.float64) + bv
    s = np.einsum("bqd,bkd->bqk", q, k) / np.sqrt(x.shape[-1])
    s = np.where(mask[:, None, :] == 0, -1e10, s)
    s -= s.max(axis=-1, keepdims=True)
    e = np.exp(s)
    a = e / e.sum(axis=-1, keepdims=True)
    hid = np.einsum("bqk,bkd->bqd", a, v)
    out = x + hid @ Wo.T.astype(np.float64) + bo
    return out.astype(np.float32)


def make_in_maps(hidden_states, mask, Wq, bq, Wk, bk, Wv, bv, Wo, bo):
    hs = np.asarray(hidden_states, dtype=np.float32)
    mask = np.asarray(mask)
    B, S, D = hs.shape
    SQ = S // 2
    KTc = (2 * KH) // P

    # fused weights (exact f32 algebra, done once on the host):
    #   scores = x_q @ (Wq^T Wk) @ x_k^T ;  (a@v)@Wo^T = a @ (x_k @ Wv^T Wo^T)
    Wg = np.asarray(Wq, np.float32).T @ np.asarray(Wk, np.float32)
    Wvo = np.asarray(Wv, np.float32).T @ np.asarray(Wo, np.float32).T
    wg8 = np.ascontiguousarray(Wg * WSCALE).astype(FP8)
    wvo8 = np.ascontiguousarray(Wvo * WSCALE).astype(FP8)
    # v-bias and o-bias act as a constant shift after the output projection:
    # fold them into the residual input (exact).
    extra = (np.asarray(Wo, np.float32) @ np.asarray(bv, np.float32)
             + np.asarray(bo, np.float32))

    # per-(batch,half) compacted key indices
    idxs = {}
    for b in range(B):
        for h in range(2):
            idx = np.nonzero(mask[b, h * SQ:(h + 1) * SQ])[0]
            if len(idx) > KH:
                return None  # caller falls back to numpy
            idxs[(b, h)] = idx

    # per-batch compacted key block + bias (shared by the two pair cores)
    xkts, mbs = {}, {}
    for b in range(B):
        x8 = hs[b].astype(FP8)
        xkT = np.zeros((D, 2 * KH), FP8)
        bias = np.full(2 * KH, np.float32(NEG_BIAS))
        for h in range(2):
            idx = idxs[(b, h)]
            xkT[:, h * KH:h * KH + len(idx)] = x8[h * SQ + idx].T
            bias[h * KH:h * KH + len(idx)] = 0.0
        bias += np.float32(EXP_OFF)
        xkts[b] = xkT
        mbs[b] = np.ascontiguousarray(bias.reshape(KTc, P).T.astype(np.float32))

    in_maps = []
    for c in range(N_CORES):
        b, h = divmod(c, 2)
        xb = hs[b]
        x8 = xb.astype(FP8)
        xqT = np.ascontiguousarray(x8[h * SQ:(h + 1) * SQ].T)
        hsc = np.ascontiguousarray(
            (xb[h * SQ:(h + 1) * SQ] + extra[None, :]).astype(np.float16))
        in_maps.append(dict(xqt=xqT, xkt=xkts[b], hs=hsc, wg=wg8,
                            wvo=wvo8, mb=mbs[b]))
    return in_maps


def assemble_output(results, B, S, D):
    SQ = S // 2
    out = np.empty((B, S, D), np.float32)
    for c in range(N_CORES):
        b, h = divmod(c, 2)
        out[b, h * SQ:(h + 1) * SQ, :] = results[c]["out"].astype(np.float32)
    return out


def kernel(hidden_states, mask, Wq, bq, Wk, bk, Wv, bv, Wo, bo):
    hs = np.asarray(hidden_states, dtype=np.float32)
    B, S, D = hs.shape
    args = dict(hidden_states=hs, mask=np.asarray(mask),
                Wq=np.asarray(Wq, np.float32), bq=np.asarray(bq, np.float32),
                Wk=np.asarray(Wk, np.float32), bk=np.asarray(bk, np.float32),
                Wv=np.asarray(Wv, np.float32), bv=np.asarray(bv, np.float32),
                Wo=np.asarray(Wo, np.float32), bo=np.asarray(bo, np.float32))
    if np.any(args["bq"]) or np.any(args["bk"]) or (S, D) != (2048, 1024):
        return _numpy_reference(**args)

    in_maps = make_in_maps(**args)
    if in_maps is None:
        return _numpy_reference(**args)
    nc = _get_program(D, S // 2)
    res = run_bass_kernel_spmd(nc, in_maps, core_ids=list(range(N_CORES)))
    return assemble_output(res.results, B, S, D)


if __name__ == "__main__":
    rng = np.random.default_rng(0)
    B, S, D = 4, 2048, 1024
    ins = dict(
        hidden_states=rng.standard_normal((B, S, D)).astype(np.float32),
        mask=rng.integers(0, 2, (B, S)).astype(np.int32),
        Wq=(rng.standard_normal((D, D)) / np.sqrt(D)).astype(np.float32),
        bq=np.zeros(D, np.float32),
        Wk=(rng.standard_normal((D, D)) / np.sqrt(D)).astype(np.float32),
        bk=np.zeros(D, np.float32),
        Wv=(rng.standard_normal((D, D)) / np.sqrt(D)).astype(np.float32),
        bv=np.zeros(D, np.float32),
        Wo=(rng.standard_normal((D, D)) / np.sqrt(D)).astype(np.float32),
        bo=np.zeros(D, np.float32),
    )
    out = kernel(**ins)
    ref = _numpy_reference(**ins)
    err = np.max(np.abs(out - ref)) / np.max(np.abs(ref))
    print("rel err vs numpy:", err)
